# revision 7
# baseline (speedup 1.0000x reference)
"""TRN2 Bass kernel for nn_Attention_20633022890922.

The reference module's einsum 'bqhk,bvhd->bqhd' contracts the attention-weight
head axis (k) and the value head axis (v) independently, so the product
factorizes into (sum_k softmax(...)) * (sum_v V) = 1 * Vsum.  The whole module
is therefore algebraically a single rank-64 linear layer:

    out = tokens @ Wv_sum @ Wo_sum + bo
      Wv_sum[h, d]  = sum_v Wv[h, v*64 + d]          (512 x 64)
      Wo_sum[d, e]  = sum_q Wo[q*64 + d, e]          (64 x 512)

(The only approximation is softmax summing to 1.0, which holds to ~1e-7 in
fp32.)  Wq / Wk cancel entirely.

Device strategy: data-parallel over the batch dim (8 batches -> 8 cores).
Per core: Y = X @ Wv_sum @ Wo_sum with X [8192, 512].  The kernel is
HBM-bound (358 GB/s/core), so every I/O tensor is fp16: X is cast+
pre-transposed on the host to hid-major [4, 128, 8192] (all device DMAs
plain contiguous), Y is stored fp16 and upcast on the host.  Weights are
single fp16 (measured end-to-end max-rel ~5e-4 vs the 2e-2 budget; the PE
quantizes operands to ~12 mantissa bits anyway).

  GEMM1 per 512-token chunk: pt[0:64] = Wv_sum.T @ X^T, 4 accumulating
        K=128 matmuls, weight-stationary-outer across each 1024-token wave
        (a stationary switch costs an array drain; reuse streams at
        N cycles/matmul).
  GEMM2 per 128-token tile: py[128 tok, 512] = tt[0:64, tile].T @ Wo_sum,
        K=64, N=512.
  PSUM->SBUF fp16 conversion copies are spread across vector/scalar/gpsimd
  so no single engine serializes the store stream.
  bias bo is all-zero per the spec; if nonzero it is added on the host
  during unsharding.
"""

import time

import numpy as np

from concourse import bacc, mybir, tile
from concourse import bass_utils

B, N_TOK, HID, EMB, NH, HD = 8, 8192, 512, 512, 8, 64
N_CORES = 8
CH = 512                      # tokens per compute chunk
WAVE = 1024                   # tokens per load wave
NCHUNK = N_TOK // CH          # 16
NWAVE = N_TOK // WAVE         # 8
CPW = WAVE // CH              # chunks per wave = 2

F32 = mybir.dt.float32
FP16 = mybir.dt.float16

_compiled = None


def _build():
    nc = bacc.Bacc(
        trn_type="TRN2", target_bir_lowering=False, debug=False, num_devices=N_CORES
    )

    # host-transposed fp16 X: [4 hid-blocks, 128 hid, 8192 tokens]
    xf_d = nc.dram_tensor("xf", [4, 128, N_TOK], FP16, kind="ExternalInput")
    # packed consts: [wv-chip 4x64 cols | wo (rows 0-63) 512 cols] fp16
    cw_d = nc.dram_tensor("cw", [128, 768], FP16, kind="ExternalInput")
    y_d = nc.dram_tensor("y", [N_TOK, HID], FP16, kind="ExternalOutput")

    with tile.TileContext(nc) as tc:
        with (
            tc.tile_pool(name="const", bufs=1) as constp,
            tc.tile_pool(name="xt", bufs=16) as xt_p,
            tc.tile_pool(name="tt", bufs=3) as tt_p,
            tc.tile_pool(name="yout", bufs=8) as y_p,
            tc.tile_pool(name="ps_t", bufs=4, space="PSUM") as ps_t,
            tc.tile_pool(name="ps_y", bufs=4, space="PSUM") as ps_y,
        ):
            cw = constp.tile([128, 768], FP16, tag="cw")
            # split const load: the first GEMM1 matmuls only need wv
            nc.scalar.dma_start(cw[:, 0:256], cw_d[:, 0:256])
            nc.scalar.dma_start(cw[:, 256:768], cw_d[:, 256:768])
            wop = cw[0:64, 256:768]

            xt_by_wave = []
            for w in range(NWAVE):
                # plain contiguous loads, one per hid-block (fine-grained
                # deps: the first GEMM1 matmuls only need block j=0)
                xt = []
                for j in range(4):
                    t = xt_p.tile([128, WAVE], FP16, tag="xt", name=f"xt{w}_{j}")
                    nc.sync.dma_start(t[:], xf_d[j, :, w * WAVE:(w + 1) * WAVE])
                    xt.append(t)
                xt_by_wave.append(xt)

            for w in range(NWAVE):
                xt = xt_by_wave[w]
                # ---- GEMM1: pt[0:64] = T^T for this chunk.  Wave 0 runs
                # chunk-major so chunk 0 finishes ASAP (the store stream is
                # the critical chain); later waves run weight-stationary-
                # outer so each stationary streams all the wave's chunks.
                pts = [ps_t.tile([64, CH], F32, tag="pt", name=f"pt{w}_{q}")
                       for q in range(CPW)]
                if w == 0:
                    for q in range(CPW):
                        for j in range(4):
                            ws = cw[:, j * 64:(j + 1) * 64]
                            nc.tensor.matmul(
                                pts[q][:], ws,
                                xt[j][:, q * CH:(q + 1) * CH],
                                start=(j == 0), stop=(j == 3),
                                skip_group_check=True,
                            )
                else:
                    for j in range(4):
                        ws = cw[:, j * 64:(j + 1) * 64]
                        for q in range(CPW):
                            nc.tensor.matmul(
                                pts[q][:], ws,
                                xt[j][:, q * CH:(q + 1) * CH],
                                start=(j == 0), stop=(j == 3),
                                skip_group_check=True,
                            )

                for q in range(CPW):
                    c = w * CPW + q
                    # ---- T^T to SBUF as fp16 for GEMM2 (only DVE/Act can
                    # read PSUM; alternate to balance)
                    tt = tt_p.tile([64, CH], FP16, tag="tt")
                    if c % 2 == 0:
                        nc.vector.tensor_copy(tt[:], pts[q][:])
                    else:
                        nc.scalar.copy(tt[:], pts[q][:])

                    # ---- GEMM2 (K=64): y[tile, :] = T @ Wo_sum
                    yo = y_p.tile([128, 4, HID], FP16, tag="yo")
                    for i in range(4):
                        py = ps_y.tile([128, HID], F32, tag="py")
                        nc.tensor.matmul(
                            py[:], tt[:, 128 * i:128 * (i + 1)], wop,
                            start=True, stop=True,
                        )
                        if i % 2 == (c % 2):
                            nc.scalar.copy(yo[:, i, :], py[:])
                        else:
                            nc.vector.tensor_copy(yo[:, i, :], py[:])

                    ydst = y_d[c * CH:(c + 1) * CH, :].rearrange(
                        "(i p) h -> p i h", p=128
                    )
                    if c < NCHUNK - 1:
                        eng = nc.sync if c % 2 == 0 else nc.scalar
                        eng.dma_start(ydst, yo[:])
                    else:
                        # final chunk: 4 small stores on both rings so the
                        # last completion receipt is short and parallel
                        for i in range(4):
                            eng = nc.sync if i % 2 == 0 else nc.scalar
                            eng.dma_start(ydst[:, i, :], yo[:, i, :])

    nc.compile()
    return nc


def _get_compiled():
    global _compiled
    if _compiled is None:
        _compiled = _build()
    return _compiled


def kernel(tokens, Wq, Wk, Wv, Wo, bo, _trace=False):
    tokens = np.asarray(tokens, dtype=np.float32)
    Wv = np.asarray(Wv, dtype=np.float32)
    Wo = np.asarray(Wo, dtype=np.float32)
    bo = np.asarray(bo, dtype=np.float32)

    # Host-side prep: fold weights, cast X to fp16 and pre-transpose it to
    # hid-major so all device DMAs are contiguous.
    wv_sum = Wv.reshape(HID, NH, HD).sum(axis=1).astype(np.float32)
    wo_sum = Wo.reshape(NH, HD, HID).sum(axis=0).astype(np.float32)
    wv16 = wv_sum.astype(np.float16)                           # [512, 64]
    wo16 = wo_sum.astype(np.float16)                           # [64, 512]
    cw = np.zeros((128, 768), dtype=np.float16)
    # stationary j: [128 hid-in-block, 64 wv cols]
    cw[:, 0:256] = wv16.reshape(4, 128, 64).transpose(1, 0, 2).reshape(128, 256)
    cw[0:64, 256:768] = wo16

    xf = tokens.astype(np.float16)           # [B, N, 512]
    # -> [B, 4 hid-blocks, 128 hid, N tokens] (host-side transpose)
    xf = np.ascontiguousarray(xf.reshape(B, N_TOK, 4, 128).transpose(0, 2, 3, 1))

    nc = _get_compiled()
    in_maps = [
        {"xf": xf[b], "cw": cw}
        for b in range(N_CORES)
    ]
    # retry once or twice on transient device flakes (rare NRT_EXEC_UNIT
    # wedges have been observed under the axon PJRT path)
    for attempt in range(3):
        try:
            res = bass_utils.run_bass_kernel_spmd(
                nc, in_maps, core_ids=list(range(N_CORES)), trace=_trace
            )
            break
        except Exception:
            if attempt == 2:
                raise
            time.sleep(20)
    out = np.stack(
        [res.results[b]["y"].astype(np.float32) for b in range(N_CORES)], axis=0
    )
    if np.any(bo):
        out += bo
    if _trace:
        return out, res
    return out


if __name__ == "__main__":
    rng = np.random.default_rng(0)
    ins = {
        "tokens": rng.standard_normal((B, N_TOK, HID)).astype(np.float32),
        "Wq": (rng.standard_normal((HID, EMB)) * 0.02).astype(np.float32),
        "Wk": (rng.standard_normal((HID, EMB)) * 0.02).astype(np.float32),
        "Wv": (rng.standard_normal((HID, HID)) * 0.02).astype(np.float32),
        "Wo": (rng.standard_normal((EMB, HID)) * 0.02).astype(np.float32),
        "bo": np.zeros((HID,), dtype=np.float32),
    }
    out = kernel(**ins)
    print(out.shape, out.dtype)
